# revision 1
# baseline (speedup 1.0000x reference)
"""MHA TRN2 kernel: folded projections; bf16 stage-P, fp8-DoubleRow scores,
f32r attention-value.

Math identical to the f32r baseline (KWq = (Wk Wq^T)-fold on the key side,
VWp = (x_kv Wv) Wp-fold so the attention-value matmul directly emits the
projected per-head partials). Precision plan, chosen from measured
per-stage error attribution vs the 2e-2 budget:

  stage P (KWq/VWp):  bf16 inputs, plain matmuls   (input-quant err ~4e-4)
  scores:             fp8e4m3 DoubleRow over channel pairs (2x ALU)
                      kwq requant ~6e-3, x_q quant ~1.3e-2
  exp -> probs:       f32 (exact), denominators consistent
  attention-value:    f32r singles (exact)
  partials out:       bf16 (~1.6e-3)

Scale plan (powers of 2, cancel exactly):
  wq_dram = at_h * 4096 (bf16), wp_dram = b_h * 512 (bf16),
  xkvt = x_kv^T / 8 (bf16)  -> kwq PSUM = 512 * (A x_kv^T)  [sigma~9, e4m3]
                               vw  PSUM =  64 * (x_kv b_h)  [f32r]
  scores PSUM = 512 * logits; exp(scale=1/512, bias=cb);
  po = 64 * partials; host divides by 64 after the den normalize.

The q-bias term bq.k and the key mask are combined into cb on the HOST.
Normalization on host: kernel ships unnormalized partials + denominators.
"""

import math
from contextlib import ExitStack
from functools import lru_cache

import numpy as np
import ml_dtypes

import concourse.tile as tile
from concourse import bacc, mybir
from concourse.bass_utils import run_bass_kernel_spmd

B, S, D, H = 4, 2048, 512, 8
NCORES = 8
MASK_NEG = -30000.0

F32 = mybir.dt.float32
F32R = mybir.dt.float32r
F8 = mybir.dt.float8e4
BF16 = mybir.dt.bfloat16
AF = mybir.ActivationFunctionType
DR = mybir.MatmulPerfMode.DoubleRow

NP_F8 = ml_dtypes.float8_e4m3
NP_BF16 = ml_dtypes.bfloat16

SC_WQ = 4096.0
SC_WP = 512.0
SC_XKV = 0.125
SC_KWQ = SC_WQ * SC_XKV  # 512
SC_VW = SC_WP * SC_XKV  # 64
EXP_SCALE = 1.0 / SC_KWQ


def _emit(nc, b_sz, s_sz, kv_tiles, rep=1):
    s_kv = max(kv_tiles) * 128
    NSB = s_sz // 512
    NC = D // 128  # 4 channel tiles

    xt_d = nc.dram_tensor("xt", [b_sz, NC, 128, s_sz], F8, kind="ExternalInput")
    xkvt_d = nc.dram_tensor("xkvt", [b_sz, NC, 128, s_kv], BF16, kind="ExternalInput")
    cb_d = nc.dram_tensor("cb", [b_sz, 128, s_kv // 128], F32, kind="ExternalInput")
    wq_d = nc.dram_tensor("wq", [D, D], BF16, kind="ExternalInput")  # at_h*SC_WQ
    wp_d = nc.dram_tensor("wp", [D, D], BF16, kind="ExternalInput")  # b_h*SC_WP
    out_d = nc.dram_tensor("out", [b_sz, NC, 128, s_sz], BF16, kind="ExternalOutput")
    den_d = nc.dram_tensor("den", [b_sz, NSB * 512], F32, kind="ExternalOutput")

    def make_groups(nt):
        widths = []
        remt = nt
        while remt > 0:
            take = min(4, remt)
            widths.append(take)
            remt -= take
        if len(widths) > 1 and widths[-1] == 1:
            widths[-2] -= 2
            widths[-1] += 2
        groups = []
        pos = 0
        for w in widths:
            groups.append(list(range(pos, pos + w)))
            pos += w
        return groups

    with tile.TileContext(nc) as tc, ExitStack() as ctx:
        ep = ctx.enter_context
        cpool = ep(tc.tile_pool(name="const", bufs=1))
        wpool = ep(tc.tile_pool(name="w", bufs=1))
        mpool = ep(tc.tile_pool(name="mask", bufs=2))
        xtqp = ep(tc.tile_pool(name="xtq", bufs=2))
        xtkp = ep(tc.tile_pool(name="xtk", bufs=2))
        vwp = ep(tc.tile_pool(name="vw", bufs=2))
        kwp = ep(tc.tile_pool(name="kw", bufs=2))
        ptp = ep(tc.tile_pool(name="pt", bufs=6))
        srp = ep(tc.tile_pool(name="sr", bufs=3))
        denp = ep(tc.tile_pool(name="den", bufs=2))
        resp = ep(tc.tile_pool(name="res", bufs=2))
        pop = ep(tc.tile_pool(name="po", bufs=4, space="PSUM"))
        psp = ep(tc.tile_pool(name="pss", bufs=4, space="PSUM"))

        ones_f = cpool.tile([128, 1], F32)
        nc.vector.memset(ones_f[:], 1.0)
        ones = cpool.tile([128, 1], F32R)
        nc.vector.tensor_copy(ones[:], ones_f[:])

        # PE warm-up: ~6us of dummy matmuls during the initial DMA wait so
        # the HAM clock-gate releases (1.2 -> 2.4 GHz) before real work.
        wupad = cpool.tile([128, 512], F32R)
        nc.vector.memset(wupad[:].bitcast(F32), 0.0)
        for _ in range(7):
            pwu = psp.tile([1, 512], F32, tag="psmall", name="warm")
            nc.tensor.matmul(pwu[:], ones[:], wupad[:], start=True, stop=True)

        wq = wpool.tile([128, NC, D], BF16)
        wp = wpool.tile([128, NC, D], BF16)

        _wloads = {
            "wq": lambda: nc.sync.dma_start(
                wq[:], wq_d.ap().rearrange("(c p) e -> p c e", p=128)
            ),
            "wp": lambda: nc.sync.dma_start(
                wp[:], wp_d.ap().rearrange("(c p) e -> p c e", p=128)
            ),
        }

        def load_weights(*names):
            for n in names:
                fn = _wloads.pop(n, None)
                if fn is not None:
                    fn()

        def prefetch_inputs(b):
            """Allocate + DMA the next batch's inputs; dispatched before the
            current batch's out-DMAs so the transfers never race stage P."""
            nt_n = kv_tiles[b]
            cb_n = mpool.tile([128, nt_n], F32, name="cb")
            xTk_n = xtkp.tile([128, NC, nt_n * 128], BF16, name="xtk")
            xTq_n = xtqp.tile([128, NC, s_sz], F8, name="xtq")
            nc.sync.dma_start(
                xTk_n[:],
                xkvt_d.ap()[b, :, :, : nt_n * 128].rearrange("c p n -> p c n"),
            )
            nc.sync.dma_start(cb_n[:], cb_d.ap()[b][:, :nt_n])
            nc.sync.dma_start(xTq_n[:], xt_d.ap()[b].rearrange("c p n -> p c n"))
            return cb_n, xTk_n, xTq_n

        prefetched = None
        batch_seq = [b for _ in range(rep) for b in range(b_sz)]
        for it, b in enumerate(batch_seq):
            nt_b = kv_tiles[b]
            kv_groups = make_groups(nt_b)

            # ---- stage P: kwq = e4m3(512*(A x_kv^T)), vw = f32r 64*(x_kv b_h)
            if prefetched is not None:
                cb, xTk, xTq = prefetched
            else:
                cb = mpool.tile([128, nt_b], F32, name="cb")
                xTk = xtkp.tile([128, NC, nt_b * 128], BF16, name="xtk")
                xTq = xtqp.tile([128, NC, s_sz], F8, name="xtq")
            vw = vwp.tile([128, nt_b, D], BF16)
            kwq = kwp.tile([128, NC, nt_b * 128], F8)
            pending_vw = []
            pending_kw = None

            def emit_kwq(n0, nw):
                pkw = [
                    pop.tile([128, 512], F32, tag="po", name=f"pkw{m}")
                    for m in range(NC)
                ]
                for e in range(NC):
                    for m in range(NC):
                        nc.tensor.matmul(
                            pkw[m][:, :nw],
                            wq[:, e, m * 128 : (m + 1) * 128],
                            xTk[:, e, n0 : n0 + nw],
                            start=(e == 0),
                            stop=(e == NC - 1),
                        )
                for m in range(NC):
                    if m % 2 == 0:
                        nc.scalar.activation(
                            kwq[:, m, n0 : n0 + nw], pkw[m][:, :nw], AF.Copy
                        )
                    else:
                        nc.vector.tensor_copy(kwq[:, m, n0 : n0 + nw], pkw[m][:, :nw])

            def emit_vwp(t):
                ps = psp.tile([128, 512], F32, tag="psmall", name="psw")
                for e in range(NC):
                    nc.tensor.matmul(
                        ps[:],
                        xTk[:, e, t * 128 : (t + 1) * 128],
                        wp[:, e, :],
                        start=(e == 0),
                        stop=(e == NC - 1),
                    )
                if t % 2 == 0:
                    nc.scalar.activation(vw[:, t, :], ps[:], AF.Copy)
                else:
                    nc.vector.tensor_copy(vw[:, t, :], ps[:])

            for gi, tiles in enumerate(kv_groups):
                last = gi == len(kv_groups) - 1
                n0 = tiles[0] * 128
                nw = len(tiles) * 128
                if prefetched is None:
                    if gi == 0:
                        # dispatch in first-consumption order: VWp(g0) runs
                        # first and walks wp channel by channel, then KWq(g0)
                        # needs wq; c0 pieces lead so the first matmul waits
                        # only on them
                        _wloads.pop("wq", None)
                        _wloads.pop("wp", None)
                        nc.sync.dma_start(
                            xTk[:, 0, n0 : n0 + nw],
                            xkvt_d.ap()[b, 0, :, n0 : n0 + nw],
                        )
                        nc.sync.dma_start(wp[:, 0, :], wp_d.ap()[0:128, :])
                        nc.sync.dma_start(
                            xTk[:, 1:NC, n0 : n0 + nw],
                            xkvt_d.ap()[b, 1:NC, :, n0 : n0 + nw].rearrange(
                                "c p n -> p c n"
                            ),
                        )
                        nc.sync.dma_start(
                            wp[:, 1:NC, :],
                            wp_d.ap()[128:, :].rearrange("(c p) e -> p c e", p=128),
                        )
                        nc.sync.dma_start(
                            wq[:], wq_d.ap().rearrange("(c p) e -> p c e", p=128)
                        )
                    else:
                        nc.sync.dma_start(
                            xTk[:, :, n0 : n0 + nw],
                            xkvt_d.ap()[b, :, :, n0 : n0 + nw].rearrange(
                                "c p n -> p c n"
                            ),
                        )
                # VWp/KWq deferred one group so the wq/wp DMAs precede
                # their first readers in program order
                vw_ready = pending_vw
                kw_ready = pending_kw
                pending_vw = list(tiles)
                pending_kw = (n0, nw)
                for t in vw_ready:
                    emit_vwp(t)
                if kw_ready is not None:
                    emit_kwq(*kw_ready)
            for t in pending_vw:
                emit_vwp(t)
            emit_kwq(*pending_kw)
            if prefetched is None:
                nc.sync.dma_start(cb[:], cb_d.ap()[b][:, :nt_b])
                nc.sync.dma_start(xTq[:], xt_d.ap()[b].rearrange("c p n -> p c n"))
            if it + 1 < len(batch_seq):
                prefetched = prefetch_inputs(batch_seq[it + 1])
            else:
                prefetched = None

            # ---- stage A: per query-block attention ----
            den_b = denp.tile([1, NSB * 512], F32)
            for sb in range(NSB):
                po = [
                    pop.tile([128, 512], F32, tag="po", name=f"po{i}")
                    for i in range(NC)
                ]
                srun = srp.tile([128, 512], F32)
                srun_r = None

                def av_group(t, ptile):
                    for m in range(NC):
                        nc.tensor.matmul(
                            po[m][:],
                            vw[:, t, m * 128 : (m + 1) * 128],
                            ptile[:],
                            start=(t == 0),
                            stop=(t == nt_b - 1),
                        )

                pend_av = []  # lag 2: av(t) emitted after scores(t+2) so the
                # exp never blocks the PE at block boundaries
                for t in range(nt_b):
                    ps = psp.tile([128, 512], F32, tag="psmall", name="pss")
                    for ci in range(NC // 2):
                        c = 2 * ci
                        nc.tensor.matmul(
                            ps[:],
                            kwq[:, c : c + 2, t * 128 : (t + 1) * 128],
                            xTq[:, c : c + 2, sb * 512 : (sb + 1) * 512],
                            start=(ci == 0),
                            stop=(ci == NC // 2 - 1),
                            perf_mode=DR,
                        )
                    if len(pend_av) >= 2:
                        av_group(*pend_av.pop(0))
                    ptile = ptp.tile([128, 512], BF16)
                    nc.scalar.activation(
                        ptile[:], ps[:], AF.Exp,
                        bias=cb[:, t : t + 1], scale=EXP_SCALE,
                    )
                    if t < nt_b - 1:
                        if t == 0:
                            nc.vector.tensor_copy(srun[:], ptile[:])
                        else:
                            nc.vector.tensor_add(srun[:], srun[:], ptile[:])
                    else:
                        srun_r = srp.tile([128, 512], F32R, name="srun_r")
                        nc.vector.tensor_add(srun_r[:], srun[:], ptile[:])
                    pend_av.append((t, ptile))
                for args in pend_av:
                    av_group(*args)
                pd = psp.tile([1, 512], F32, tag="psmall", name="pd")
                nc.tensor.matmul(pd[:], ones[:], srun_r[:], start=True, stop=True)

                # ship unnormalized partials (transposed, bf16) + denominators.
                # res copies all go on DVE: an ACT copy here would delay the
                # next block's exp in the strict-FIFO Scalar queue, stalling
                # its first AV matmul (measured 1.9us/block).
                res = resp.tile([128, NC, 512], BF16)
                for m in range(NC):
                    nc.vector.tensor_copy(res[:, m, :], po[m][:])
                    if m % 2 == 1:  # ship halves so transfer overlaps copies
                        nc.sync.dma_start(
                            out_d.ap()[
                                b, m - 1 : m + 1, :, sb * 512 : (sb + 1) * 512
                            ].rearrange("c p n -> p c n"),
                            res[:, m - 1 : m + 1, :],
                        )
                nc.scalar.activation(
                    den_b[:, sb * 512 : (sb + 1) * 512], pd[:], AF.Copy
                )
            nc.sync.dma_start(den_d.ap()[b : b + 1], den_b[:])


@lru_cache(maxsize=4)
def _build(b_sz, s_sz, kv_tiles, rep=1):
    nc = bacc.Bacc("TRN2", target_bir_lowering=False, debug=False)
    _emit(nc, b_sz, s_sz, kv_tiles, rep=rep)
    nc.compile()
    return nc


def _q8(a):
    return np.clip(a, -240.0, 240.0).astype(NP_F8)


KV_CAP = 1024  # device handles <=8 kv tiles; the ragged overflow beyond the
# cap (a handful of keys when ~50% of S=2048 survive the mask) is blended in
# exactly on the host - a full extra 128-tile on device would be ~90% padding.


def _prep_inputs(x, mask, Wq, bq, Wk, bk, Wv, bv, Wp, bp):
    b_sz, s_sz, _ = x.shape
    nc_ = D // 128
    x = np.asarray(x, dtype=np.float32)
    m = np.asarray(mask).reshape(b_sz, s_sz)
    counts = [int((m[b] != 0).sum()) for b in range(b_sz)]
    kv_tiles = tuple(
        max(2, -(-min(c, KV_CAP) // 128)) for c in counts
    )
    s_kv = max(kv_tiles) * 128
    nt_kv = s_kv // 128
    x_kv = np.zeros((b_sz, s_kv, D), dtype=np.float32)
    moff = np.full((b_sz, s_kv), np.float32(MASK_NEG), dtype=np.float32)
    x_tails = []
    for b in range(b_sz):
        idx = np.nonzero(m[b])[0]
        n_dev = min(len(idx), KV_CAP)
        x_kv[b, :n_dev] = x[b, idx[:n_dev]]
        moff[b, :n_dev] = 0.0
        x_tails.append(x[b, idx[n_dev:]].astype(np.float64))
    xt = np.ascontiguousarray(
        _q8(x).transpose(0, 2, 1).reshape(b_sz, nc_, 128, s_sz)
    )
    xkvt = np.ascontiguousarray(
        (x_kv * SC_XKV).astype(NP_BF16).transpose(0, 2, 1).reshape(
            b_sz, nc_, 128, s_kv
        )
    )

    sc = 1.0 / math.sqrt(D)
    x_kv64 = x_kv.astype(np.float64)
    x64 = x.astype(np.float64)
    in_maps = []
    tails = []  # per head: (num [b,S,D] f32, den [b,S] f32) for capped keys
    for h in range(NCORES):
        wq64 = np.asarray(Wq[h], dtype=np.float64) * sc
        wk64 = np.asarray(Wk[h], dtype=np.float64)
        wv64 = np.asarray(Wv[h], dtype=np.float64)
        wph64 = np.asarray(Wp[h * D : (h + 1) * D, :], dtype=np.float64)
        at64 = wk64 @ wq64.T
        bh64 = wv64 @ wph64
        at_h = (at64 * SC_WQ).astype(NP_BF16)
        b_h = (bh64 * SC_WP).astype(NP_BF16)
        bqk = x_kv64 @ (wk64 @ (np.asarray(bq[h], np.float64) * sc))
        cb_h = moff + bqk.astype(np.float32)
        cb_h = np.ascontiguousarray(
            cb_h.reshape(b_sz, nt_kv, 128).transpose(0, 2, 1)
        )
        # exact host blend for the tail keys beyond KV_CAP
        tnum = np.zeros((b_sz, s_sz, D), dtype=np.float32)
        tden = np.zeros((b_sz, s_sz), dtype=np.float32)
        bqk_w = wk64 @ (np.asarray(bq[h], np.float64) * sc)
        for b in range(b_sz):
            xt_b = x_tails[b]
            if xt_b.shape[0] == 0:
                continue
            logits = (x64[b] @ at64.T) @ xt_b.T + (xt_b @ bqk_w)[None, :]
            p = np.exp(logits)
            tnum[b] = (p @ (xt_b @ bh64)).astype(np.float32)
            tden[b] = p.sum(axis=1).astype(np.float32)
        tails.append((tnum, tden))
        in_maps.append(
            {
                "xt": xt,
                "xkvt": xkvt,
                "cb": cb_h,
                "wq": np.ascontiguousarray(at_h),
                "wp": np.ascontiguousarray(b_h),
            }
        )
    bv64 = np.asarray(bv, dtype=np.float64)
    wp64 = np.asarray(Wp, dtype=np.float64)
    bp_eff = np.asarray(bp, dtype=np.float64).copy()
    for h in range(NCORES):
        bp_eff += bv64[h] @ wp64[h * D : (h + 1) * D, :]
    global _TAILS
    _TAILS = tails
    return in_maps, bp_eff.astype(np.float32), kv_tiles


_TAILS = None


def combine_results(results, bp_eff, b_sz, s_sz):
    """Host: blend tail keys, normalize by denominators, sum heads."""
    acc = np.zeros((b_sz, s_sz, D), dtype=np.float64)
    for h in range(NCORES):
        o = np.asarray(results[h]["out"], dtype=np.float64).reshape(b_sz, D, s_sz)
        den = np.asarray(results[h]["den"], dtype=np.float64).reshape(b_sz, s_sz)
        num = o.transpose(0, 2, 1) / SC_VW
        if _TAILS is not None:
            tnum, tden = _TAILS[h]
            num = num + tnum
            den = den + tden
        acc += num / den[:, :, None]
    out = acc + bp_eff
    return out.astype(np.float32)


def kernel(x, mask, Wq, bq, Wk, bk, Wv, bv, Wp, bp):
    x = np.asarray(x)
    b_sz, s_sz, _ = x.shape
    in_maps, bp_eff, kv_tiles = _prep_inputs(x, mask, Wq, bq, Wk, bk, Wv, bv, Wp, bp)
    nc = _build(b_sz, s_sz, kv_tiles)
    res = run_bass_kernel_spmd(nc, in_maps, list(range(NCORES)))
    return combine_results(res.results, bp_eff, b_sz, s_sz)

